# revision 50
# baseline (speedup 1.0000x reference)
"""Trainium2 Bass kernel for nn_AttentionBlock (b=4, c=512, h=w=64).

Sharding: 8 cores = (batch 0..3) x (sequence half 0..1). Each core receives
its batch's x [512, 4096] ROTATED so that the core's query half occupies
local columns 0:2048 (attention is permutation-invariant over keys, and
groupnorm stats are order-invariant, so one SPMD program serves all cores).

Per-core pipeline (fp8e4 + DoubleRow on the PE, [128,1024] "super" PSUM
tiles spanning two banks so every PSUM consumer op covers two matmul
outputs at once — halves the per-instruction overhead on ACT/DVE):
  A) x loaded ONCE into resident SBUF tiles [128, 2048] x 8; groupnorm
     stats split across DVE (bn_stats, cts 1-3) and ACT (Identity/Square
     accum_out, ct 0) + indicator matmuls for the group reduce/broadcast.
  B) normalize x from SBUF into fp8 pair tiles [128, 2, 1024] on GPSIMD
     (1-input ops are line-rate there; DVE/ACT stay free for PSUM work),
     then QKV as fp8 DoubleRow matmuls into supers: K/Q converted by DVE
     scalar_tensor_tensor (x1/16 + per-pair broadcast bias), V by ACT
     Identity (x1/16), each one [128,1024] op per super.
  C) per 512-query chunk: scores super = two S^T tiles (4 DR matmuls),
     ONE exp ACT [128,1024] (scale=1/sqrt(c), bias=-1.5; shift cancels in
     softmax, keeps E under fp8e4 max 240) -> fp8 E pair tiles; U
     accumulates in two supers over 16 key-tile pairs; Z on DVE (fp8 E
     adds into f32 [128,2,512]), folded + column-summed/broadcast via a
     (1/8)-valued f32r matmul (folds the x8 attn prescale), attn = U*(8/Z)
     via rbb broadcast to both super halves; proj DR matmuls into the
     scores ring; post-proj DVE stt applies 1/128 + pair-broadcast bias,
     residual added from the resident x tiles.  The per-qc tail (Z fold,
     attn, proj, store) is deferred into the NEXT qc's score stream so the
     PE never idles on the serial Z chain.
HBM traffic: x read once (8MB), weights ~1MB, out 4MB. No DRAM scratch.
"""

import os
import sys
from contextlib import ExitStack

for _p in ("/opt/trn_rl_repo", "/root/.axon_site/_ro/trn_rl_repo"):
    if os.path.isdir(_p) and _p not in sys.path:
        sys.path.insert(0, _p)

import numpy as np
import ml_dtypes

import concourse.bass as bass
import concourse.tile as tile
from concourse.tile_rust import add_dep_helper
from concourse import bacc, mybir
from concourse.bass_utils import run_bass_kernel_spmd

F32 = mybir.dt.float32
F32R = mybir.dt.float32r
BF16 = mybir.dt.bfloat16
F8 = mybir.dt.float8e4
NP8 = ml_dtypes.float8_e4m3
NPBF = ml_dtypes.bfloat16
ALU = mybir.AluOpType
ACT = mybir.ActivationFunctionType
DR = mybir.MatmulPerfMode.DoubleRow

N_CORES = 8
C = 512          # channels
N = 4096         # h*w
NQ = 2048        # queries per core
CT = C // 128    # 4 channel tiles
NCHUNK = N // 512   # 8 column chunks
QCHUNK = NQ // 512  # 4 query chunks per core
MT = N // 128    # 32 key tiles
PRS = MT // 2    # 16 key-tile pairs
GSIZE = 16       # channels per group
EPS = 1e-5
WSCALE = 16.0    # host-side fp8 weight prescale
ASCALE = 8.0     # attn prescale (folded into the Z broadcast matmul)
ESHIFT = -1.5    # exp shift; cancels in softmax
SCALE_QK = 1.0 / float(np.sqrt(np.float32(C)))
NORM_ON_GPSIMD = True


def build_module(reps: int = 1):
    nc = bacc.Bacc("TRN2", target_bir_lowering=False, debug=False,
                   num_devices=N_CORES)

    xin = nc.dram_tensor("xin", [C, N], BF16, kind="ExternalInput").ap()
    w8 = nc.dram_tensor("w8", [C, 3 * C], F8, kind="ExternalInput").ap()
    wpd = nc.dram_tensor("wpd", [C, C], F8, kind="ExternalInput").ap()
    cvec = nc.dram_tensor("cvec", [128, 28], F32, kind="ExternalInput").ap()
    indb = nc.dram_tensor("indb", [8, 128], F32, kind="ExternalInput").ap()
    out = nc.dram_tensor("out", [C, NQ], F32, kind="ExternalOutput").ap()

    with tile.TileContext(nc) as tc, \
            nc.allow_low_precision(reason="fp8 attention by design"):
        for rep in range(reps):
            _emit_body(tc, rep, xin, w8, wpd, cvec, indb, out)
    nc.compile()
    return nc


def _emit_body(tc, rep, xin, w8, wpd, cvec, indb, out):
    nc = tc.nc
    norm_eng = nc.gpsimd if NORM_ON_GPSIMD else nc.vector
    with ExitStack() as ctx:
        # ---- persistent pools ----
        const = ctx.enter_context(tc.tile_pool(name=f"const{rep}", bufs=1))
        # one pool per resident-x tile: consumers then wait only on that
        # tile's two DMAs instead of the whole 16-DMA batch
        xpools = [ctx.enter_context(tc.tile_pool(name=f"xres{rep}_{i}",
                                                 bufs=1)) for i in range(8)]
        kpool = ctx.enter_context(tc.tile_pool(name=f"kbuf{rep}", bufs=1))
        vpool = ctx.enter_context(tc.tile_pool(name=f"vbuf{rep}", bufs=1))
        qpool = ctx.enter_context(tc.tile_pool(name=f"qbuf{rep}", bufs=1))
        wpool = ctx.enter_context(tc.tile_pool(name=f"wgt{rep}", bufs=1))
        statp = ctx.enter_context(tc.tile_pool(name=f"stat{rep}", bufs=1))

        # resident x: xa[ct*2 + jj] = [128, 2048] bf16 (cols jj*2048...)
        xa = [xpools[i].tile([128, 2048], BF16, name=f"xa{i}", tag=f"xa{i}")
              for i in range(8)]
        # fp8 K: pair p (channels 256p..256p+255), column chunk j
        K_f8 = [[kpool.tile([128, 2, 512], F8, name=f"K{p}_{j}",
                            tag=f"K{p}_{j}") for j in range(NCHUNK)]
                for p in range(2)]
        # fp8 V^T: key-tile pair pr, free = 512 channels
        V_f8 = [vpool.tile([128, 2, 512], F8, name=f"V{pr}", tag=f"V{pr}")
                for pr in range(PRS)]
        # fp8 Q: query chunk qc, channel pair p
        Q_f8 = [[qpool.tile([128, 2, 512], F8, name=f"Q{qc}_{p}",
                            tag=f"Q{qc}_{p}") for p in range(2)]
                for qc in range(QCHUNK)]
        # fp8 weights: qkv pair tiles and proj pair tiles
        w3 = [wpool.tile([128, 2, 3 * C], F8, name=f"w3_{p}", tag=f"w3_{p}")
              for p in range(2)]
        wp8 = [wpool.tile([128, 2, C], F8, name=f"wp{p}", tag=f"wp{p}")
               for p in range(2)]

        cvec_t = const.tile([128, 28], F32, name="cvec", tag="cvec")
        indb_t = const.tile([8, 128], F32, name="indb", tag="indb")
        ones_mat_f = const.tile([128, 128], F32, name="onemf", tag="onemf")
        nc.vector.memset(ones_mat_f[:], 1.0 / ASCALE)
        ones_mat = const.tile([128, 128], F32R, name="onem", tag="onem")
        nc.vector.tensor_copy(ones_mat[:], ones_mat_f[:])
        eps_t = const.tile([128, 1], F32, name="epst", tag="epst")
        nc.vector.memset(eps_t[:], EPS)
        esh_t = const.tile([128, 1], F32, name="esht", tag="esht")
        nc.vector.memset(esh_t[:], ESHIFT)
        # cvec layout: [bq x4][bk x4][bp x4][gnw x4][gnb x4][indr x8]
        bq_t = [cvec_t[:, ct:ct + 1] for ct in range(CT)]
        bp_t = [cvec_t[:, 8 + ct:9 + ct] for ct in range(CT)]
        gnw_t = [cvec_t[:, 12 + ct:13 + ct] for ct in range(CT)]
        gnb_t = [cvec_t[:, 16 + ct:17 + ct] for ct in range(CT)]
        indr_t = cvec_t[:, 20:28]
        # pair bias tiles [128, 2, 1] for free-dim broadcast over supers
        bq2 = [const.tile([128, 2, 1], F32, name=f"bq2_{p}", tag=f"bq2_{p}")
               for p in range(2)]
        bk2 = [const.tile([128, 2, 1], F32, name=f"bk2_{p}", tag=f"bk2_{p}")
               for p in range(2)]
        bp2 = [const.tile([128, 2, 1], F32, name=f"bp2_{p}", tag=f"bp2_{p}")
               for p in range(2)]


        scale_ca = statp.tile([128, CT], F32, name="sca", tag="sca")
        bias_ca = statp.tile([128, CT], F32, name="bca", tag="bca")
        scale_c = [scale_ca[:, ct:ct + 1] for ct in range(CT)]
        bias_c = [bias_ca[:, ct:ct + 1] for ct in range(CT)]

        # ================= Phase A: load x + groupnorm statistics ========
        with ExitStack() as pa:
            scpa = pa.enter_context(tc.tile_pool(name=f"sca{rep}", bufs=2))
            tmpa = pa.enter_context(tc.tile_pool(name=f"tmpa{rep}", bufs=2))
            psa = pa.enter_context(
                tc.tile_pool(name=f"psa{rep}", bufs=2, space="PSUM"))

            # stats split 2.5/1.5: DVE bn_stats takes ct1, ct2, ct3-jj0;
            # ACT (Identity/Square accum) takes ct0 and ct3-jj1.  x tiles
            # go out FIRST on both queues, DVE's earliest.
            stats = {ct: statp.tile([128, NCHUNK, 6], F32, name=f"st{ct}",
                                    tag=f"st{ct}") for ct in (1, 2, 3)}
            sacc = {ct: statp.tile([128, 4], F32, name=f"sa{ct}",
                                   tag=f"sa{ct}") for ct in (0, 3)}
            # t12a layout: cols [mean x4 | E[x^2] x4]
            t12a = tmpa.tile([128, 2 * CT], F32, name="t12a", tag="t12a")
            last_a_load = None
            # (ct, jj, queue): ACT's first tile heads the sync queue so
            # both stats engines start as early as possible
            load_order = [(0, 0, "s"), (3, 0, "g"), (1, 0, "s"),
                          (1, 1, "g"), (2, 0, "s"), (0, 1, "g"),
                          (2, 1, "s"), (3, 1, "g")]
            for ct, jj, q in load_order:
                xt = xa[ct * 2 + jj]
                src = xin[ct * 128:(ct + 1) * 128,
                          jj * 2048:(jj + 1) * 2048]
                if q == "s":
                    last_a_load = nc.sync.dma_start(xt[:], src)
                else:
                    nc.gpsimd.dma_start(xt[:], src)
                dve_side = ct in (1, 2) or (ct == 3 and jj == 0)
                if dve_side:
                    for kk in range(4):
                        nc.vector.bn_stats(
                            out=stats[ct][:, 4 * jj + kk, :],
                            in_=xt[:, kk * 512:(kk + 1) * 512])
                else:
                    cj = 2 * jj if ct == 0 else 0
                    scr = scpa.tile([128, 2048], BF16, name="scr",
                                    tag="scr")
                    nc.scalar.activation(
                        out=scr[:], in_=xt[:], func=ACT.Identity,
                        accum_out=sacc[ct][:, cj:cj + 1])
                    nc.scalar.activation(
                        out=scr[:], in_=xt[:], func=ACT.Square,
                        accum_out=sacc[ct][:, cj + 1:cj + 2])
            # constants follow the x tiles on the sync queue
            nc.sync.dma_start(cvec_t[:], cvec)
            nc.sync.dma_start(indb_t[:], indb)
            for p in range(2):
                nc.vector.tensor_copy(bq2[p][:], cvec_t[:, 2 * p:2 * p + 2])
                nc.vector.tensor_copy(bk2[p][:],
                                      cvec_t[:, 4 + 2 * p:6 + 2 * p])
                nc.vector.tensor_copy(bp2[p][:],
                                      cvec_t[:, 8 + 2 * p:10 + 2 * p])
            # per-ct totals into t12a
            for ct in range(CT):
                if ct in (1, 2):
                    mv = tmpa.tile([128, 2], F32, name="mv", tag="mv")
                    nc.vector.bn_aggr(out=mv[:], in_=stats[ct][:])
                    nc.vector.tensor_copy(t12a[:, ct:ct + 1], mv[:, 0:1])
                    nc.vector.tensor_mul(t12a[:, CT + ct:CT + ct + 1],
                                         mv[:, 0:1], mv[:, 0:1])
                    nc.vector.tensor_add(t12a[:, CT + ct:CT + ct + 1],
                                         t12a[:, CT + ct:CT + ct + 1],
                                         mv[:, 1:2])
                elif ct == 0:
                    sa = sacc[0]
                    nc.vector.tensor_add(t12a[:, 0:1], sa[:, 0:1],
                                         sa[:, 2:3])
                    nc.vector.tensor_add(t12a[:, CT:CT + 1], sa[:, 1:2],
                                         sa[:, 3:4])
                    nc.vector.tensor_scalar_mul(t12a[:, 0:1],
                                                t12a[:, 0:1], 1.0 / N)
                    nc.vector.tensor_scalar_mul(t12a[:, CT:CT + 1],
                                                t12a[:, CT:CT + 1], 1.0 / N)
                else:  # ct == 3: DVE half (jj0) + ACT sums (jj1)
                    mv = tmpa.tile([128, 2], F32, name="mv", tag="mv")
                    nc.vector.bn_aggr(out=mv[:], in_=stats[3][:, 0:4, :])
                    sa = sacc[3]
                    # mean = mv.mean/2 + S1/N
                    nc.vector.tensor_scalar_mul(t12a[:, 3:4],
                                                mv[:, 0:1], 0.5)
                    nc.vector.scalar_tensor_tensor(
                        out=t12a[:, 3:4], in0=sa[:, 0:1], scalar=1.0 / N,
                        in1=t12a[:, 3:4], op0=ALU.mult, op1=ALU.add)
                    # EX2 = (mv.var + mv.mean^2)/2 + S2/N
                    ex2 = tmpa.tile([128, 1], F32, name="ex2", tag="ex2")
                    nc.vector.tensor_mul(ex2[:], mv[:, 0:1], mv[:, 0:1])
                    nc.vector.tensor_add(ex2[:], ex2[:], mv[:, 1:2])
                    nc.vector.tensor_scalar_mul(t12a[:, CT + 3:CT + 4],
                                                ex2[:], 0.5)
                    nc.vector.scalar_tensor_tensor(
                        out=t12a[:, CT + 3:CT + 4], in0=sa[:, 1:2],
                        scalar=1.0 / N, in1=t12a[:, CT + 3:CT + 4],
                        op0=ALU.mult, op1=ALU.add)
            # one group reduce + broadcast for ALL channel tiles:
            # [8, 8] = indr^T @ t12a ; [128, 8] = indb^T @ g12
            gps = psa.tile([8, 2 * CT], F32, name="gps", tag="gps")
            nc.tensor.matmul(gps[:], indr_t, t12a[:], start=True, stop=True)
            g12 = tmpa.tile([8, 2 * CT], F32, name="g12", tag="g12")
            nc.vector.tensor_copy(g12[:], gps[:])
            cps = psa.tile([128, 2 * CT], F32, name="cps", tag="cps")
            nc.tensor.matmul(cps[:], indb_t[:], g12[:], start=True, stop=True)
            cs = tmpa.tile([128, 2 * CT], F32, name="cs", tag="cs")
            nc.vector.tensor_copy(cs[:], cps[:])
            # var = E[x^2] - mean^2 ; rstd = 1/sqrt(var+eps)  (all 4 tiles)
            var_t = tmpa.tile([128, CT], F32, name="var", tag="var")
            nc.vector.tensor_mul(var_t[:], cs[:, 0:CT], cs[:, 0:CT])
            nc.vector.tensor_sub(var_t[:], cs[:, CT:2 * CT], var_t[:])
            sq_t = tmpa.tile([128, CT], F32, name="sq", tag="sq")
            nc.scalar.activation(out=sq_t[:], in_=var_t[:],
                                 func=ACT.Sqrt, bias=eps_t[:], scale=1.0)
            rstd_t = tmpa.tile([128, CT], F32, name="rstd", tag="rstd")
            nc.vector.reciprocal(rstd_t[:], sq_t[:])
            nc.vector.tensor_mul(scale_ca[:], rstd_t[:], cvec_t[:, 12:16])
            mt_t = tmpa.tile([128, CT], F32, name="mt", tag="mt")
            nc.vector.tensor_mul(mt_t[:], cs[:, 0:CT], scale_ca[:])
            nc.vector.tensor_sub(bias_ca[:], cvec_t[:, 16:20], mt_t[:])

        # weight loads (after the x DMAs on the sync ring)
        for p in range(2):
            for s in range(2):
                r0 = p * 256 + s * 128
                wd = nc.sync.dma_start(w3[p][:, s, :], w8[r0:r0 + 128, :])
                add_dep_helper(wd.ins, last_a_load.ins, sync=True,
                               reason="x loads first on the sync ring")
                wd = nc.sync.dma_start(wp8[p][:, s, :], wpd[r0:r0 + 128, :])
                add_dep_helper(wd.ins, last_a_load.ins, sync=True,
                               reason="x loads first on the sync ring")

        # ================= Phase B: normalize + QKV (fp8 DR supers) ======
        with ExitStack() as pb:
            xbp = pb.enter_context(tc.tile_pool(name=f"xb{rep}", bufs=2))
            psb = pb.enter_context(
                tc.tile_pool(name=f"psb{rep}", bufs=3, space="PSUM"))

            for jp in range(NCHUNK // 2):
                xf8 = []
                for p in range(2):
                    xt = xbp.tile([128, 2, 1024], F8, name=f"xf{p}",
                                  tag=f"xf{p}")
                    for s in range(2):
                        ct = 2 * p + s
                        src = xa[ct * 2 + jp // 2][
                            :, (jp % 2) * 1024:(jp % 2) * 1024 + 1024]
                        # first chunk is latency-critical (and GPSIMD pays
                        # a ~6us ucode IRAM load on its first tensor op):
                        # run it on DVE + ACT, GPSIMD handles the rest
                        if jp == 0 and p == 0:
                            nc.vector.tensor_scalar(
                                out=xt[:, s, :], in0=src,
                                scalar1=scale_c[ct],
                                scalar2=bias_c[ct],
                                op0=ALU.mult, op1=ALU.add)
                        elif jp == 0:
                            nc.scalar.activation(
                                out=xt[:, s, :], in_=src,
                                func=ACT.Identity,
                                bias=bias_c[ct], scale=scale_c[ct])
                        else:
                            norm_eng.tensor_scalar(
                                out=xt[:, s, :], in0=src,
                                scalar1=scale_c[ct],
                                scalar2=bias_c[ct],
                                op0=ALU.mult, op1=ALU.add)
                    xf8.append(xt)

                for jh in range(2):
                    j = jp * 2 + jh
                    xn = [xf8[p][:, :, jh * 512:(jh + 1) * 512]
                          for p in range(2)]
                    # K supers: halves (ot=2h, 2h+1) -> K_f8[h][j]
                    for h in range(2):
                        ks = psb.tile([128, 1024], F32, name="sup",
                                      tag="sup")
                        for s in range(2):
                            ot = 2 * h + s
                            for p in range(2):
                                nc.tensor.matmul(
                                    ks[:, s * 512:(s + 1) * 512],
                                    w3[p][:, :,
                                          C + ot * 128:C + (ot + 1) * 128],
                                    xn[p], start=(p == 0), stop=(p == 1),
                                    perf_mode=DR)
                        nc.vector.scalar_tensor_tensor(
                            out=K_f8[h][j][:], in0=ks[:],
                            scalar=1.0 / WSCALE,
                            in1=bk2[h][:].to_broadcast((128, 2, 512)),
                            op0=ALU.mult, op1=ALU.add)
                    # V supers: halves mt=(4j+2i, 4j+2i+1) -> V_f8[2j+i]
                    for i in range(2):
                        pr = 2 * j + i
                        vs = psb.tile([128, 1024], F32, name="sup",
                                      tag="sup")
                        for s in range(2):
                            mti = 2 * i + s
                            for p in range(2):
                                nc.tensor.matmul(
                                    vs[:, s * 512:(s + 1) * 512],
                                    xn[p][:, :, mti * 128:(mti + 1) * 128],
                                    w3[p][:, :, 2 * C:3 * C],
                                    start=(p == 0), stop=(p == 1),
                                    perf_mode=DR)
                        if j >= NCHUNK - 2:
                            # keep ACT's FIFO clear near the end of phase
                            # B so the first exp isn't queued behind it
                            nc.vector.tensor_scalar_mul(
                                V_f8[pr][:], vs[:], 1.0 / WSCALE)
                        else:
                            nc.scalar.activation(
                                out=V_f8[pr][:], in_=vs[:],
                                func=ACT.Identity, scale=1.0 / WSCALE)
                    # Q supers (only local columns 0:2048 are queries)
                    if j < QCHUNK:
                        for h in range(2):
                            qs = psb.tile([128, 1024], F32, name="sup",
                                          tag="sup")
                            for s in range(2):
                                ot = 2 * h + s
                                for p in range(2):
                                    nc.tensor.matmul(
                                        qs[:, s * 512:(s + 1) * 512],
                                        w3[p][:, :,
                                              ot * 128:(ot + 1) * 128],
                                        xn[p], start=(p == 0), stop=(p == 1),
                                        perf_mode=DR)
                            nc.vector.scalar_tensor_tensor(
                                out=Q_f8[j][h][:], in0=qs[:],
                                scalar=1.0 / WSCALE,
                                in1=bq2[h][:].to_broadcast((128, 2, 512)),
                                op0=ALU.mult, op1=ALU.add)

        # ================= Phase C: attention + proj (fp8 DR supers) =====
        with ExitStack() as pc:
            epool = pc.enter_context(tc.tile_pool(name=f"e{rep}", bufs=10))
            apool = pc.enter_context(tc.tile_pool(name=f"at{rep}", bufs=2))
            outp = pc.enter_context(tc.tile_pool(name=f"out{rep}", bufs=3))
            miscp = pc.enter_context(tc.tile_pool(name=f"mi{rep}", bufs=2))
            ps_s = pc.enter_context(
                tc.tile_pool(name=f"pss{rep}", bufs=2, space="PSUM"))
            ps_u = pc.enter_context(
                tc.tile_pool(name=f"psu{rep}", bufs=1, space="PSUM"))

            pending_zb = None
            pending_proj = None
            for qc in range(QCHUNK):
                e_tiles = {}

                def scores_pair(pr, qc=qc, e_tiles=e_tiles):
                    ss = ps_s.tile([128, 1024], F32, name="s", tag="s")
                    for i2 in range(2):
                        mt = 2 * pr + i2
                        for p in range(2):
                            nc.tensor.matmul(
                                ss[:, i2 * 512:(i2 + 1) * 512],
                                K_f8[p][mt // 4][
                                    :, :, (mt % 4) * 128:(mt % 4 + 1) * 128],
                                Q_f8[qc][p][:], start=(p == 0), stop=(p == 1),
                                perf_mode=DR)
                    e = epool.tile([128, 2, 512], F8, name="e", tag="e")
                    nc.scalar.activation(
                        out=e[:], in_=ss[:], func=ACT.Exp,
                        bias=esh_t[:], scale=SCALE_QK)
                    e_tiles[pr] = e

                # 8 score pairs head start; the previous chunk's tail is
                # threaded between them so the PE never waits on the
                # serial Z/attn chain
                for pr0 in range(4):
                    scores_pair(pr0)
                if pending_zb is not None:
                    pending_zb()
                    pending_zb = None
                for pr0 in range(4, 8):
                    scores_pair(pr0)
                if pending_proj is not None:
                    pending_proj()
                    pending_proj = None

                u = [ps_u.tile([128, 1024], F32, name=f"u{h}", tag=f"u{h}")
                     for h in range(2)]
                # Z split across two engines: DVE takes odd key-tile
                # pairs, GPSIMD even ones — neither chain lags the PE
                zaccA = miscp.tile([128, 2, 512], F32, name="zaA",
                                   tag="zaA")
                zaccB = miscp.tile([128, 2, 512], F32, name="zaB",
                                   tag="zaB")

                def pv(pr, u=u, zaccA=zaccA, zaccB=zaccB, e_tiles=e_tiles):
                    e = e_tiles.pop(pr)
                    for ct in range(CT):
                        nc.tensor.matmul(
                            u[ct // 2][:, (ct % 2) * 512:(ct % 2 + 1) * 512],
                            V_f8[pr][:, :, ct * 128:(ct + 1) * 128],
                            e[:], start=(pr == 0), stop=(pr == PRS - 1),
                            perf_mode=DR)
                    if pr % 2 == 0:
                        if pr == 0:
                            nc.gpsimd.tensor_copy(zaccB[:], e[:])
                        else:
                            nc.gpsimd.tensor_add(zaccB[:], zaccB[:], e[:])
                    else:
                        if pr == 1:
                            nc.vector.tensor_copy(zaccA[:], e[:])
                        else:
                            nc.vector.tensor_add(zaccA[:], zaccA[:], e[:])

                for pr in range(PRS):
                    if pr + 8 < PRS:
                        scores_pair(pr + 8)
                    pv(pr)

                # fold Z immediately (DVE/GPSIMD only, no PE)
                zhB = miscp.tile([128, 512], F32, name="zhB", tag="zhB")
                nc.gpsimd.tensor_add(zhB[:], zaccB[:, 0, :],
                                     zaccB[:, 1, :])
                zh = miscp.tile([128, 512], F32R, name="zh", tag="zh")
                nc.vector.tensor_add(zh[:], zaccA[:, 0, :], zaccA[:, 1, :])
                nc.vector.tensor_add(zh[:], zh[:], zhB[:])

                def tail_zb(qc=qc, u=u, zh=zh, state=None):
                    # column-sum + broadcast via the (1/ASCALE)-valued
                    # matmul; rbb = ASCALE / Z; attn = U * rbb in fp8
                    zsup = ps_s.tile([128, 1024], F32, name="s", tag="s")
                    nc.tensor.matmul(zsup[:, 0:512], ones_mat[:], zh[:],
                                     start=True, stop=True)
                    rbb = miscp.tile([128, 1, 512], F32, name="rb",
                                     tag="rb")
                    nc.vector.reciprocal_approx_fast(rbb[:], zsup[:, 0:512])
                    attn8 = [apool.tile([128, 2, 512], F8, name=f"a{p}",
                                        tag=f"a{p}") for p in range(2)]
                    for p in range(2):
                        nc.vector.tensor_mul(
                            attn8[p][:], u[p][:],
                            rbb[:].to_broadcast((128, 2, 512)))
                    state["attn8"] = attn8

                def tail_proj(qc=qc, state=None):
                    attn8 = state["attn8"]
                    # proj PSUM reuses the (drained) U banks
                    pps = [ps_u.tile([128, 1024], F32, name=f"u{h}",
                                     tag=f"u{h}") for h in range(2)]
                    for p in range(2):
                        for h in range(2):
                            for s in range(2):
                                ot = 2 * h + s
                                nc.tensor.matmul(
                                    pps[h][:, s * 512:(s + 1) * 512],
                                    wp8[p][:, :, ot * 128:(ot + 1) * 128],
                                    attn8[p][:], start=(p == 0),
                                    stop=(p == 1), perf_mode=DR)
                    for h in range(2):
                        t_o = outp.tile([128, 1024], F32, name="out",
                                        tag="out")
                        if h == 1:
                            # h1 via two per-half ACT ops, in parallel
                            # with h0's DVE work
                            for s in range(2):
                                nc.scalar.activation(
                                    out=t_o[:, s * 512:(s + 1) * 512],
                                    in_=pps[h][:, s * 512:(s + 1) * 512],
                                    func=ACT.Identity,
                                    bias=bp_t[2 * h + s],
                                    scale=1.0 / (WSCALE * ASCALE))
                        else:
                            nc.vector.scalar_tensor_tensor(
                                out=t_o[:], in0=pps[h][:],
                                scalar=1.0 / (WSCALE * ASCALE),
                                in1=bp2[h][:].to_broadcast((128, 2, 512)),
                                op0=ALU.mult, op1=ALU.add)
                        for s in range(2):
                            ot = 2 * h + s
                            nc.vector.tensor_add(
                                t_o[:, s * 512:(s + 1) * 512],
                                t_o[:, s * 512:(s + 1) * 512],
                                xa[ot * 2][:, qc * 512:(qc + 1) * 512])
                            # final chunk: split stores across both DMA
                            # queues to shorten the drain
                            dq = (nc.gpsimd if (qc == QCHUNK - 1 and h == 1)
                                  else nc.sync)
                            dq.dma_start(
                                out[ot * 128:(ot + 1) * 128,
                                    qc * 512:(qc + 1) * 512],
                                t_o[:, s * 512:(s + 1) * 512])

                def make_pending(tz=tail_zb, tp=tail_proj):
                    st = {}

                    def pz():
                        tz(state=st)

                    def pp_():
                        tp(state=st)
                    return pz, pp_

                pending_zb, pending_proj = make_pending()
            pending_zb()
            pending_proj()


# ---------------- host-side sharding / gather ----------------

_CACHED_NC = None


def _get_nc():
    global _CACHED_NC
    if _CACHED_NC is None:
        _CACHED_NC = build_module(reps=1)
    return _CACHED_NC


def _make_in_maps(x, gn_w, gn_b, qkv_w, qkv_b, proj_w, proj_b):
    b, c, h, w = x.shape
    n = h * w
    assert (b, c, n) == (4, C, N)
    xr = np.ascontiguousarray(x.reshape(b, c, n)).astype(np.float32)
    xr16 = xr.astype(NPBF)

    # fp8 weights, prescaled x16.  No 1/sqrt(c) folding: that lives in the
    # exp activation's scale.
    w8_h = np.ascontiguousarray(
        np.concatenate([qkv_w[0:c].T, qkv_w[c:2 * c].T, qkv_w[2 * c:3 * c].T],
                       axis=1) * WSCALE).astype(NP8)
    wp_h = np.ascontiguousarray(proj_w.T * WSCALE).astype(NP8)

    bq_h = np.asarray(qkv_b[0:c], np.float32).reshape(CT, 128)
    bk_h = np.asarray(qkv_b[c:2 * c], np.float32).reshape(CT, 128)
    # v-bias folded through the projection:  proj(attn + bv) =
    # proj(attn) + proj_w @ bv, so it lands in the proj bias.
    bp_eff = (np.asarray(proj_b, np.float64)
              + np.asarray(proj_w, np.float64) @ np.asarray(
                  qkv_b[2 * c:3 * c], np.float64)).astype(np.float32)
    bp_h = bp_eff.reshape(CT, 128)
    gnw_h = np.asarray(gn_w, np.float32).reshape(CT, 128)
    gnb_h = np.asarray(gn_b, np.float32).reshape(CT, 128)
    pidx = np.arange(128)
    indr_h = (pidx[:, None] // GSIZE == np.arange(8)[None, :]).astype(
        np.float32) / GSIZE
    indb_h = (np.arange(8)[:, None] == pidx[None, :] // GSIZE).astype(
        np.float32)
    cvec_h = np.zeros((128, 28), np.float32)
    for ct in range(CT):
        cvec_h[:, ct] = bq_h[ct]
        cvec_h[:, 4 + ct] = bk_h[ct]
        cvec_h[:, 8 + ct] = bp_h[ct]
        cvec_h[:, 12 + ct] = gnw_h[ct]
        cvec_h[:, 16 + ct] = gnb_h[ct]
    cvec_h[:, 20:28] = indr_h

    shared = dict(w8=w8_h, wpd=wp_h, cvec=cvec_h, indb=indb_h)
    in_maps = []
    for core in range(N_CORES):
        bi, half = core // 2, core % 2
        xb = xr16[bi]
        if half:
            xb = np.ascontiguousarray(
                np.concatenate([xb[:, NQ:], xb[:, :NQ]], axis=1))
        in_maps.append({"xin": xb, **shared})
    return in_maps


def kernel(x, gn_w, gn_b, qkv_w, qkv_b, proj_w, proj_b):
    nc = _get_nc()
    in_maps = _make_in_maps(x, gn_w, gn_b, qkv_w, qkv_b, proj_w, proj_b)
    res = run_bass_kernel_spmd(nc, in_maps, list(range(N_CORES)))
    b, c, h, w = x.shape
    out_full = np.empty((b, C, N), dtype=np.float32)
    for core in range(N_CORES):
        bi, half = core // 2, core % 2
        out_full[bi, :, half * NQ:(half + 1) * NQ] = res.results[core]["out"]
    return out_full.reshape(b, c, h, w)


# revision 51
# speedup vs baseline: 1.0210x; 1.0210x over previous
"""Trainium2 Bass kernel for nn_AttentionBlock (b=4, c=512, h=w=64).

Sharding: 8 cores = (batch 0..3) x (sequence half 0..1). Each core receives
its batch's x [512, 4096] ROTATED so that the core's query half occupies
local columns 0:2048 (attention is permutation-invariant over keys, and
groupnorm stats are order-invariant, so one SPMD program serves all cores).

Per-core pipeline (fp8e4 + DoubleRow on the PE, [128,1024] "super" PSUM
tiles spanning two banks so every PSUM consumer op covers two matmul
outputs at once — halves the per-instruction overhead on ACT/DVE):
  A) x loaded ONCE into resident SBUF tiles [128, 2048] x 8; groupnorm
     stats split across DVE (bn_stats, cts 1-3) and ACT (Identity/Square
     accum_out, ct 0) + indicator matmuls for the group reduce/broadcast.
  B) normalize x from SBUF into fp8 pair tiles [128, 2, 1024] on GPSIMD
     (1-input ops are line-rate there; DVE/ACT stay free for PSUM work),
     then QKV as fp8 DoubleRow matmuls into supers: K/Q converted by DVE
     scalar_tensor_tensor (x1/16 + per-pair broadcast bias), V by ACT
     Identity (x1/16), each one [128,1024] op per super.
  C) per 512-query chunk: scores super = two S^T tiles (4 DR matmuls),
     ONE exp ACT [128,1024] (scale=1/sqrt(c), bias=-1.5; shift cancels in
     softmax, keeps E under fp8e4 max 240) -> fp8 E pair tiles; U
     accumulates in two supers over 16 key-tile pairs; Z on DVE (fp8 E
     adds into f32 [128,2,512]), folded + column-summed/broadcast via a
     (1/8)-valued f32r matmul (folds the x8 attn prescale), attn = U*(8/Z)
     via rbb broadcast to both super halves; proj DR matmuls into the
     scores ring; post-proj DVE stt applies 1/128 + pair-broadcast bias,
     residual added from the resident x tiles.  The per-qc tail (Z fold,
     attn, proj, store) is deferred into the NEXT qc's score stream so the
     PE never idles on the serial Z chain.
HBM traffic: x read once (8MB), weights ~1MB, out 4MB. No DRAM scratch.
"""

import os
import sys
from contextlib import ExitStack

for _p in ("/opt/trn_rl_repo", "/root/.axon_site/_ro/trn_rl_repo"):
    if os.path.isdir(_p) and _p not in sys.path:
        sys.path.insert(0, _p)

import numpy as np
import ml_dtypes

import concourse.bass as bass
import concourse.tile as tile
from concourse.tile_rust import add_dep_helper
from concourse import bacc, mybir
from concourse.bass_utils import run_bass_kernel_spmd

F32 = mybir.dt.float32
F32R = mybir.dt.float32r
BF16 = mybir.dt.bfloat16
F8 = mybir.dt.float8e4
NP8 = ml_dtypes.float8_e4m3
NPBF = ml_dtypes.bfloat16
ALU = mybir.AluOpType
ACT = mybir.ActivationFunctionType
DR = mybir.MatmulPerfMode.DoubleRow

N_CORES = 8
C = 512          # channels
N = 4096         # h*w
NQ = 2048        # queries per core
CT = C // 128    # 4 channel tiles
NCHUNK = N // 512   # 8 column chunks
QCHUNK = NQ // 512  # 4 query chunks per core
MT = N // 128    # 32 key tiles
PRS = MT // 2    # 16 key-tile pairs
GSIZE = 16       # channels per group
EPS = 1e-5
WSCALE = 16.0    # host-side fp8 weight prescale
ASCALE = 8.0     # attn prescale (folded into the Z broadcast matmul)
ESHIFT = -1.5    # exp shift; cancels in softmax
SCALE_QK = 1.0 / float(np.sqrt(np.float32(C)))
NORM_ON_GPSIMD = True


def build_module(reps: int = 1):
    nc = bacc.Bacc("TRN2", target_bir_lowering=False, debug=False,
                   num_devices=N_CORES)

    xin = nc.dram_tensor("xin", [C, N], BF16, kind="ExternalInput").ap()
    w8 = nc.dram_tensor("w8", [C, 3 * C], F8, kind="ExternalInput").ap()
    wpd = nc.dram_tensor("wpd", [C, C], F8, kind="ExternalInput").ap()
    cvec = nc.dram_tensor("cvec", [128, 28], F32, kind="ExternalInput").ap()
    indb = nc.dram_tensor("indb", [8, 128], F32, kind="ExternalInput").ap()
    out = nc.dram_tensor("out", [C, NQ], F32, kind="ExternalOutput").ap()

    with tile.TileContext(nc) as tc, \
            nc.allow_low_precision(reason="fp8 attention by design"):
        for rep in range(reps):
            _emit_body(tc, rep, xin, w8, wpd, cvec, indb, out)
    nc.compile()
    return nc


def _emit_body(tc, rep, xin, w8, wpd, cvec, indb, out):
    nc = tc.nc
    norm_eng = nc.gpsimd if NORM_ON_GPSIMD else nc.vector
    with ExitStack() as ctx:
        # ---- persistent pools ----
        const = ctx.enter_context(tc.tile_pool(name=f"const{rep}", bufs=1))
        # one pool per resident-x tile: consumers then wait only on that
        # tile's two DMAs instead of the whole 16-DMA batch
        xpools = [ctx.enter_context(tc.tile_pool(name=f"xres{rep}_{i}",
                                                 bufs=1)) for i in range(8)]
        kpool = ctx.enter_context(tc.tile_pool(name=f"kbuf{rep}", bufs=1))
        vpool = ctx.enter_context(tc.tile_pool(name=f"vbuf{rep}", bufs=1))
        qpool = ctx.enter_context(tc.tile_pool(name=f"qbuf{rep}", bufs=1))
        wpool = ctx.enter_context(tc.tile_pool(name=f"wgt{rep}", bufs=1))
        statp = ctx.enter_context(tc.tile_pool(name=f"stat{rep}", bufs=1))

        # resident x: xa[ct*2 + jj] = [128, 2048] bf16 (cols jj*2048...)
        xa = [xpools[i].tile([128, 2048], BF16, name=f"xa{i}", tag=f"xa{i}")
              for i in range(8)]
        # fp8 K: pair p (channels 256p..256p+255), column chunk j
        K_f8 = [[kpool.tile([128, 2, 512], F8, name=f"K{p}_{j}",
                            tag=f"K{p}_{j}") for j in range(NCHUNK)]
                for p in range(2)]
        # fp8 V^T: key-tile pair pr, free = 512 channels
        V_f8 = [vpool.tile([128, 2, 512], F8, name=f"V{pr}", tag=f"V{pr}")
                for pr in range(PRS)]
        # fp8 Q: query chunk qc, channel pair p
        Q_f8 = [[qpool.tile([128, 2, 512], F8, name=f"Q{qc}_{p}",
                            tag=f"Q{qc}_{p}") for p in range(2)]
                for qc in range(QCHUNK)]
        # fp8 weights: qkv pair tiles and proj pair tiles
        w3 = [wpool.tile([128, 2, 3 * C], F8, name=f"w3_{p}", tag=f"w3_{p}")
              for p in range(2)]
        wp8 = [wpool.tile([128, 2, C], F8, name=f"wp{p}", tag=f"wp{p}")
               for p in range(2)]

        cvec_t = const.tile([128, 28], F32, name="cvec", tag="cvec")
        indb_t = const.tile([8, 128], F32, name="indb", tag="indb")
        ones_mat_f = const.tile([128, 128], F32, name="onemf", tag="onemf")
        nc.vector.memset(ones_mat_f[:], 1.0 / ASCALE)
        ones_mat = const.tile([128, 128], F32R, name="onem", tag="onem")
        nc.vector.tensor_copy(ones_mat[:], ones_mat_f[:])
        eps_t = const.tile([128, 1], F32, name="epst", tag="epst")
        nc.vector.memset(eps_t[:], EPS)
        esh_t = const.tile([128, 1], F32, name="esht", tag="esht")
        nc.vector.memset(esh_t[:], ESHIFT)
        # cvec layout: [bq x4][bk x4][bp x4][gnw x4][gnb x4][indr x8]
        bq_t = [cvec_t[:, ct:ct + 1] for ct in range(CT)]
        bp_t = [cvec_t[:, 8 + ct:9 + ct] for ct in range(CT)]
        gnw_t = [cvec_t[:, 12 + ct:13 + ct] for ct in range(CT)]
        gnb_t = [cvec_t[:, 16 + ct:17 + ct] for ct in range(CT)]
        indr_t = cvec_t[:, 20:28]
        # pair bias tiles [128, 2, 1] for free-dim broadcast over supers
        bq2 = [const.tile([128, 2, 1], F32, name=f"bq2_{p}", tag=f"bq2_{p}")
               for p in range(2)]
        bk2 = [const.tile([128, 2, 1], F32, name=f"bk2_{p}", tag=f"bk2_{p}")
               for p in range(2)]
        bp2 = [const.tile([128, 2, 1], F32, name=f"bp2_{p}", tag=f"bp2_{p}")
               for p in range(2)]


        scale_ca = statp.tile([128, CT], F32, name="sca", tag="sca")
        bias_ca = statp.tile([128, CT], F32, name="bca", tag="bca")
        scale_c = [scale_ca[:, ct:ct + 1] for ct in range(CT)]
        bias_c = [bias_ca[:, ct:ct + 1] for ct in range(CT)]

        # ================= Phase A: load x + groupnorm statistics ========
        with ExitStack() as pa:
            scpa = pa.enter_context(tc.tile_pool(name=f"sca{rep}", bufs=2))
            tmpa = pa.enter_context(tc.tile_pool(name=f"tmpa{rep}", bufs=2))
            psa = pa.enter_context(
                tc.tile_pool(name=f"psa{rep}", bufs=2, space="PSUM"))

            # stats split 2.5/1.5: DVE bn_stats takes ct1, ct2, ct3-jj0;
            # ACT (Identity/Square accum) takes ct0 and ct3-jj1.  x tiles
            # go out FIRST on both queues, DVE's earliest.
            stats = {ct: statp.tile([128, NCHUNK, 6], F32, name=f"st{ct}",
                                    tag=f"st{ct}") for ct in (1, 2, 3)}
            sacc = {ct: statp.tile([128, 4], F32, name=f"sa{ct}",
                                   tag=f"sa{ct}") for ct in (0, 3)}
            # t12a layout: cols [mean x4 | E[x^2] x4]
            t12a = tmpa.tile([128, 2 * CT], F32, name="t12a", tag="t12a")
            last_a_load = None
            # (ct, jj, queue): ACT's first tile heads the sync queue so
            # both stats engines start as early as possible
            load_order = [(0, 0, "s"), (3, 0, "g"), (1, 0, "s"),
                          (1, 1, "g"), (2, 0, "s"), (0, 1, "g"),
                          (2, 1, "s"), (3, 1, "g")]
            for ct, jj, q in load_order:
                xt = xa[ct * 2 + jj]
                src = xin[ct * 128:(ct + 1) * 128,
                          jj * 2048:(jj + 1) * 2048]
                if q == "s":
                    last_a_load = nc.sync.dma_start(xt[:], src)
                else:
                    nc.gpsimd.dma_start(xt[:], src)
                dve_side = ct in (1, 2) or (ct == 3 and jj == 0)
                if dve_side:
                    for kk in range(4):
                        nc.vector.bn_stats(
                            out=stats[ct][:, 4 * jj + kk, :],
                            in_=xt[:, kk * 512:(kk + 1) * 512])
                else:
                    cj = 2 * jj if ct == 0 else 0
                    scr = scpa.tile([128, 2048], BF16, name="scr",
                                    tag="scr")
                    nc.scalar.activation(
                        out=scr[:], in_=xt[:], func=ACT.Identity,
                        accum_out=sacc[ct][:, cj:cj + 1])
                    nc.scalar.activation(
                        out=scr[:], in_=xt[:], func=ACT.Square,
                        accum_out=sacc[ct][:, cj + 1:cj + 2])
            # constants follow the x tiles on the sync queue
            nc.sync.dma_start(cvec_t[:], cvec)
            nc.sync.dma_start(indb_t[:], indb)
            for p in range(2):
                nc.vector.tensor_copy(bq2[p][:], cvec_t[:, 2 * p:2 * p + 2])
                nc.vector.tensor_copy(bk2[p][:],
                                      cvec_t[:, 4 + 2 * p:6 + 2 * p])
                nc.vector.tensor_copy(bp2[p][:],
                                      cvec_t[:, 8 + 2 * p:10 + 2 * p])
            # per-ct totals into t12a
            for ct in range(CT):
                if ct in (1, 2):
                    mv = tmpa.tile([128, 2], F32, name="mv", tag="mv")
                    nc.vector.bn_aggr(out=mv[:], in_=stats[ct][:])
                    nc.vector.tensor_copy(t12a[:, ct:ct + 1], mv[:, 0:1])
                    nc.vector.tensor_mul(t12a[:, CT + ct:CT + ct + 1],
                                         mv[:, 0:1], mv[:, 0:1])
                    nc.vector.tensor_add(t12a[:, CT + ct:CT + ct + 1],
                                         t12a[:, CT + ct:CT + ct + 1],
                                         mv[:, 1:2])
                elif ct == 0:
                    sa = sacc[0]
                    nc.vector.tensor_add(t12a[:, 0:1], sa[:, 0:1],
                                         sa[:, 2:3])
                    nc.vector.tensor_add(t12a[:, CT:CT + 1], sa[:, 1:2],
                                         sa[:, 3:4])
                    nc.vector.tensor_scalar_mul(t12a[:, 0:1],
                                                t12a[:, 0:1], 1.0 / N)
                    nc.vector.tensor_scalar_mul(t12a[:, CT:CT + 1],
                                                t12a[:, CT:CT + 1], 1.0 / N)
                else:  # ct == 3: DVE half (jj0) + ACT sums (jj1)
                    mv = tmpa.tile([128, 2], F32, name="mv", tag="mv")
                    nc.vector.bn_aggr(out=mv[:], in_=stats[3][:, 0:4, :])
                    sa = sacc[3]
                    # mean = mv.mean/2 + S1/N
                    nc.vector.tensor_scalar_mul(t12a[:, 3:4],
                                                mv[:, 0:1], 0.5)
                    nc.vector.scalar_tensor_tensor(
                        out=t12a[:, 3:4], in0=sa[:, 0:1], scalar=1.0 / N,
                        in1=t12a[:, 3:4], op0=ALU.mult, op1=ALU.add)
                    # EX2 = (mv.var + mv.mean^2)/2 + S2/N
                    ex2 = tmpa.tile([128, 1], F32, name="ex2", tag="ex2")
                    nc.vector.tensor_mul(ex2[:], mv[:, 0:1], mv[:, 0:1])
                    nc.vector.tensor_add(ex2[:], ex2[:], mv[:, 1:2])
                    nc.vector.tensor_scalar_mul(t12a[:, CT + 3:CT + 4],
                                                ex2[:], 0.5)
                    nc.vector.scalar_tensor_tensor(
                        out=t12a[:, CT + 3:CT + 4], in0=sa[:, 1:2],
                        scalar=1.0 / N, in1=t12a[:, CT + 3:CT + 4],
                        op0=ALU.mult, op1=ALU.add)
            # one group reduce + broadcast for ALL channel tiles:
            # [8, 8] = indr^T @ t12a ; [128, 8] = indb^T @ g12
            gps = psa.tile([8, 2 * CT], F32, name="gps", tag="gps")
            nc.tensor.matmul(gps[:], indr_t, t12a[:], start=True, stop=True)
            g12 = tmpa.tile([8, 2 * CT], F32, name="g12", tag="g12")
            nc.vector.tensor_copy(g12[:], gps[:])
            cps = psa.tile([128, 2 * CT], F32, name="cps", tag="cps")
            nc.tensor.matmul(cps[:], indb_t[:], g12[:], start=True, stop=True)
            cs = tmpa.tile([128, 2 * CT], F32, name="cs", tag="cs")
            nc.vector.tensor_copy(cs[:], cps[:])
            # var = E[x^2] - mean^2 ; rstd = 1/sqrt(var+eps)  (all 4 tiles)
            var_t = tmpa.tile([128, CT], F32, name="var", tag="var")
            nc.vector.tensor_mul(var_t[:], cs[:, 0:CT], cs[:, 0:CT])
            nc.vector.tensor_sub(var_t[:], cs[:, CT:2 * CT], var_t[:])
            sq_t = tmpa.tile([128, CT], F32, name="sq", tag="sq")
            nc.scalar.activation(out=sq_t[:], in_=var_t[:],
                                 func=ACT.Sqrt, bias=eps_t[:], scale=1.0)
            rstd_t = tmpa.tile([128, CT], F32, name="rstd", tag="rstd")
            nc.vector.reciprocal(rstd_t[:], sq_t[:])
            nc.vector.tensor_mul(scale_ca[:], rstd_t[:], cvec_t[:, 12:16])
            mt_t = tmpa.tile([128, CT], F32, name="mt", tag="mt")
            nc.vector.tensor_mul(mt_t[:], cs[:, 0:CT], scale_ca[:])
            nc.vector.tensor_sub(bias_ca[:], cvec_t[:, 16:20], mt_t[:])

        # weight loads (after the x DMAs on the sync ring)
        for p in range(2):
            for s in range(2):
                r0 = p * 256 + s * 128
                wd = nc.sync.dma_start(w3[p][:, s, :], w8[r0:r0 + 128, :])
                add_dep_helper(wd.ins, last_a_load.ins, sync=True,
                               reason="x loads first on the sync ring")
                wd = nc.sync.dma_start(wp8[p][:, s, :], wpd[r0:r0 + 128, :])
                add_dep_helper(wd.ins, last_a_load.ins, sync=True,
                               reason="x loads first on the sync ring")

        # ================= Phase B: normalize + QKV (fp8 DR supers) ======
        with ExitStack() as pb:
            xbp = pb.enter_context(tc.tile_pool(name=f"xb{rep}", bufs=2))
            psb = pb.enter_context(
                tc.tile_pool(name=f"psb{rep}", bufs=3, space="PSUM"))

            for jp in range(NCHUNK // 2):
                xf8 = []
                for p in range(2):
                    xt = xbp.tile([128, 2, 1024], F8, name=f"xf{p}",
                                  tag=f"xf{p}")
                    for s in range(2):
                        ct = 2 * p + s
                        src = xa[ct * 2 + jp // 2][
                            :, (jp % 2) * 1024:(jp % 2) * 1024 + 1024]
                        # first chunk is latency-critical (and GPSIMD pays
                        # a ~6us ucode IRAM load on its first tensor op):
                        # run it on DVE + ACT, GPSIMD handles the rest
                        if jp == 0 and p == 0:
                            nc.vector.tensor_scalar(
                                out=xt[:, s, :], in0=src,
                                scalar1=scale_c[ct],
                                scalar2=bias_c[ct],
                                op0=ALU.mult, op1=ALU.add)
                        elif jp == 0:
                            nc.scalar.activation(
                                out=xt[:, s, :], in_=src,
                                func=ACT.Identity,
                                bias=bias_c[ct], scale=scale_c[ct])
                        else:
                            norm_eng.tensor_scalar(
                                out=xt[:, s, :], in0=src,
                                scalar1=scale_c[ct],
                                scalar2=bias_c[ct],
                                op0=ALU.mult, op1=ALU.add)
                    xf8.append(xt)

                for jh in range(2):
                    j = jp * 2 + jh
                    xn = [xf8[p][:, :, jh * 512:(jh + 1) * 512]
                          for p in range(2)]
                    # K supers: halves (ot=2h, 2h+1) -> K_f8[h][j]
                    for h in range(2):
                        ks = psb.tile([128, 1024], F32, name="sup",
                                      tag="sup")
                        for s in range(2):
                            ot = 2 * h + s
                            for p in range(2):
                                nc.tensor.matmul(
                                    ks[:, s * 512:(s + 1) * 512],
                                    w3[p][:, :,
                                          C + ot * 128:C + (ot + 1) * 128],
                                    xn[p], start=(p == 0), stop=(p == 1),
                                    perf_mode=DR)
                        nc.vector.scalar_tensor_tensor(
                            out=K_f8[h][j][:], in0=ks[:],
                            scalar=1.0 / WSCALE,
                            in1=bk2[h][:].to_broadcast((128, 2, 512)),
                            op0=ALU.mult, op1=ALU.add)
                    # V supers: halves mt=(4j+2i, 4j+2i+1) -> V_f8[2j+i]
                    for i in range(2):
                        pr = 2 * j + i
                        vs = psb.tile([128, 1024], F32, name="sup",
                                      tag="sup")
                        for s in range(2):
                            mti = 2 * i + s
                            for p in range(2):
                                nc.tensor.matmul(
                                    vs[:, s * 512:(s + 1) * 512],
                                    xn[p][:, :, mti * 128:(mti + 1) * 128],
                                    w3[p][:, :, 2 * C:3 * C],
                                    start=(p == 0), stop=(p == 1),
                                    perf_mode=DR)
                        if j >= NCHUNK - 2:
                            # keep ACT's FIFO clear near the end of phase
                            # B so the first exp isn't queued behind it
                            nc.vector.tensor_scalar_mul(
                                V_f8[pr][:], vs[:], 1.0 / WSCALE)
                        else:
                            nc.scalar.activation(
                                out=V_f8[pr][:], in_=vs[:],
                                func=ACT.Identity, scale=1.0 / WSCALE)
                    # Q supers (only local columns 0:2048 are queries)
                    if j < QCHUNK:
                        for h in range(2):
                            qs = psb.tile([128, 1024], F32, name="sup",
                                          tag="sup")
                            for s in range(2):
                                ot = 2 * h + s
                                for p in range(2):
                                    nc.tensor.matmul(
                                        qs[:, s * 512:(s + 1) * 512],
                                        w3[p][:, :,
                                              ot * 128:(ot + 1) * 128],
                                        xn[p], start=(p == 0), stop=(p == 1),
                                        perf_mode=DR)
                            nc.vector.scalar_tensor_tensor(
                                out=Q_f8[j][h][:], in0=qs[:],
                                scalar=1.0 / WSCALE,
                                in1=bq2[h][:].to_broadcast((128, 2, 512)),
                                op0=ALU.mult, op1=ALU.add)

        # ================= Phase C: attention + proj (fp8 DR supers) =====
        with ExitStack() as pc:
            epool = pc.enter_context(tc.tile_pool(name=f"e{rep}", bufs=10))
            apool = pc.enter_context(tc.tile_pool(name=f"at{rep}", bufs=2))
            outp = pc.enter_context(tc.tile_pool(name=f"out{rep}", bufs=3))
            miscp = pc.enter_context(tc.tile_pool(name=f"mi{rep}", bufs=2))
            ps_s = pc.enter_context(
                tc.tile_pool(name=f"pss{rep}", bufs=2, space="PSUM"))
            ps_u = pc.enter_context(
                tc.tile_pool(name=f"psu{rep}", bufs=1, space="PSUM"))

            pending_zb = None
            pending_proj = None
            for qc in range(QCHUNK):
                e_tiles = {}

                def scores_pair(pr, qc=qc, e_tiles=e_tiles):
                    ss = ps_s.tile([128, 1024], F32, name="s", tag="s")
                    for i2 in range(2):
                        mt = 2 * pr + i2
                        for p in range(2):
                            nc.tensor.matmul(
                                ss[:, i2 * 512:(i2 + 1) * 512],
                                K_f8[p][mt // 4][
                                    :, :, (mt % 4) * 128:(mt % 4 + 1) * 128],
                                Q_f8[qc][p][:], start=(p == 0), stop=(p == 1),
                                perf_mode=DR)
                    e = epool.tile([128, 2, 512], F8, name="e", tag="e")
                    nc.scalar.activation(
                        out=e[:], in_=ss[:], func=ACT.Exp,
                        bias=esh_t[:], scale=SCALE_QK)
                    e_tiles[pr] = e

                # 8 score pairs head start; the previous chunk's tail is
                # threaded between them so the PE never waits on the
                # serial Z/attn chain
                for pr0 in range(4):
                    scores_pair(pr0)
                if pending_zb is not None:
                    pending_zb()
                    pending_zb = None
                for pr0 in range(4, 8):
                    scores_pair(pr0)
                if pending_proj is not None:
                    pending_proj()
                    pending_proj = None

                u = [ps_u.tile([128, 1024], F32, name=f"u{h}", tag=f"u{h}")
                     for h in range(2)]
                # Z split across two engines: DVE takes odd key-tile
                # pairs, GPSIMD even ones — neither chain lags the PE
                zaccA = miscp.tile([128, 2, 512], F32, name="zaA",
                                   tag="zaA")
                zaccB = miscp.tile([128, 2, 512], F32, name="zaB",
                                   tag="zaB")

                def pv(pr, u=u, zaccA=zaccA, zaccB=zaccB, e_tiles=e_tiles):
                    e = e_tiles.pop(pr)
                    for ct in range(CT):
                        nc.tensor.matmul(
                            u[ct // 2][:, (ct % 2) * 512:(ct % 2 + 1) * 512],
                            V_f8[pr][:, :, ct * 128:(ct + 1) * 128],
                            e[:], start=(pr == 0), stop=(pr == PRS - 1),
                            perf_mode=DR)
                    if pr % 2 == 0:
                        if pr == 0:
                            nc.gpsimd.tensor_copy(zaccB[:], e[:])
                        else:
                            nc.gpsimd.tensor_add(zaccB[:], zaccB[:], e[:])
                    else:
                        if pr == 1:
                            nc.vector.tensor_copy(zaccA[:], e[:])
                        else:
                            nc.vector.tensor_add(zaccA[:], zaccA[:], e[:])

                for pr in range(PRS):
                    if pr + 8 < PRS:
                        scores_pair(pr + 8)
                    pv(pr)

                # fold Z immediately (DVE/GPSIMD only, no PE)
                zhB = miscp.tile([128, 512], F32, name="zhB", tag="zhB")
                nc.gpsimd.tensor_add(zhB[:], zaccB[:, 0, :],
                                     zaccB[:, 1, :])
                zh = miscp.tile([128, 512], F32R, name="zh", tag="zh")
                nc.vector.tensor_add(zh[:], zaccA[:, 0, :], zaccA[:, 1, :])
                nc.vector.tensor_add(zh[:], zh[:], zhB[:])

                def tail_zb(qc=qc, u=u, zh=zh, state=None):
                    # column-sum + broadcast via the (1/ASCALE)-valued
                    # matmul; rbb = ASCALE / Z; attn = U * rbb in fp8
                    zsup = ps_s.tile([128, 1024], F32, name="s", tag="s")
                    nc.tensor.matmul(zsup[:, 0:512], ones_mat[:], zh[:],
                                     start=True, stop=True)
                    rbb = miscp.tile([128, 1, 512], F32, name="rb",
                                     tag="rb")
                    nc.vector.reciprocal_approx_fast(rbb[:], zsup[:, 0:512])
                    attn8 = [apool.tile([128, 2, 512], F8, name=f"a{p}",
                                        tag=f"a{p}") for p in range(2)]
                    for p in range(2):
                        nc.vector.tensor_mul(
                            attn8[p][:], u[p][:],
                            rbb[:].to_broadcast((128, 2, 512)))
                    state["attn8"] = attn8

                def tail_proj(qc=qc, state=None):
                    attn8 = state["attn8"]
                    # proj PSUM reuses the (drained) U banks
                    pps = [ps_u.tile([128, 1024], F32, name=f"u{h}",
                                     tag=f"u{h}") for h in range(2)]
                    for p in range(2):
                        for h in range(2):
                            for s in range(2):
                                ot = 2 * h + s
                                nc.tensor.matmul(
                                    pps[h][:, s * 512:(s + 1) * 512],
                                    wp8[p][:, :, ot * 128:(ot + 1) * 128],
                                    attn8[p][:], start=(p == 0),
                                    stop=(p == 1), perf_mode=DR)
                    for h in range(2):
                        t_o = outp.tile([128, 1024], F32, name="out",
                                        tag="out")
                        if qc == QCHUNK - 1 and h == 1:
                            # final chunk h1: two per-half ACT ops run in
                            # parallel with h0's DVE work
                            for s in range(2):
                                nc.scalar.activation(
                                    out=t_o[:, s * 512:(s + 1) * 512],
                                    in_=pps[h][:, s * 512:(s + 1) * 512],
                                    func=ACT.Identity,
                                    bias=bp_t[2 * h + s],
                                    scale=1.0 / (WSCALE * ASCALE))
                        else:
                            nc.vector.scalar_tensor_tensor(
                                out=t_o[:], in0=pps[h][:],
                                scalar=1.0 / (WSCALE * ASCALE),
                                in1=bp2[h][:].to_broadcast((128, 2, 512)),
                                op0=ALU.mult, op1=ALU.add)
                        for s in range(2):
                            ot = 2 * h + s
                            nc.vector.tensor_add(
                                t_o[:, s * 512:(s + 1) * 512],
                                t_o[:, s * 512:(s + 1) * 512],
                                xa[ot * 2][:, qc * 512:(qc + 1) * 512])
                            # final chunk: split stores across both DMA
                            # queues to shorten the drain
                            dq = (nc.gpsimd if (qc == QCHUNK - 1 and h == 1)
                                  else nc.sync)
                            dq.dma_start(
                                out[ot * 128:(ot + 1) * 128,
                                    qc * 512:(qc + 1) * 512],
                                t_o[:, s * 512:(s + 1) * 512])

                def make_pending(tz=tail_zb, tp=tail_proj):
                    st = {}

                    def pz():
                        tz(state=st)

                    def pp_():
                        tp(state=st)
                    return pz, pp_

                pending_zb, pending_proj = make_pending()
            pending_zb()
            pending_proj()


# ---------------- host-side sharding / gather ----------------

_CACHED_NC = None


def _get_nc():
    global _CACHED_NC
    if _CACHED_NC is None:
        _CACHED_NC = build_module(reps=1)
    return _CACHED_NC


def _make_in_maps(x, gn_w, gn_b, qkv_w, qkv_b, proj_w, proj_b):
    b, c, h, w = x.shape
    n = h * w
    assert (b, c, n) == (4, C, N)
    xr = np.ascontiguousarray(x.reshape(b, c, n)).astype(np.float32)
    xr16 = xr.astype(NPBF)

    # fp8 weights, prescaled x16.  No 1/sqrt(c) folding: that lives in the
    # exp activation's scale.
    w8_h = np.ascontiguousarray(
        np.concatenate([qkv_w[0:c].T, qkv_w[c:2 * c].T, qkv_w[2 * c:3 * c].T],
                       axis=1) * WSCALE).astype(NP8)
    wp_h = np.ascontiguousarray(proj_w.T * WSCALE).astype(NP8)

    bq_h = np.asarray(qkv_b[0:c], np.float32).reshape(CT, 128)
    bk_h = np.asarray(qkv_b[c:2 * c], np.float32).reshape(CT, 128)
    # v-bias folded through the projection:  proj(attn + bv) =
    # proj(attn) + proj_w @ bv, so it lands in the proj bias.
    bp_eff = (np.asarray(proj_b, np.float64)
              + np.asarray(proj_w, np.float64) @ np.asarray(
                  qkv_b[2 * c:3 * c], np.float64)).astype(np.float32)
    bp_h = bp_eff.reshape(CT, 128)
    gnw_h = np.asarray(gn_w, np.float32).reshape(CT, 128)
    gnb_h = np.asarray(gn_b, np.float32).reshape(CT, 128)
    pidx = np.arange(128)
    indr_h = (pidx[:, None] // GSIZE == np.arange(8)[None, :]).astype(
        np.float32) / GSIZE
    indb_h = (np.arange(8)[:, None] == pidx[None, :] // GSIZE).astype(
        np.float32)
    cvec_h = np.zeros((128, 28), np.float32)
    for ct in range(CT):
        cvec_h[:, ct] = bq_h[ct]
        cvec_h[:, 4 + ct] = bk_h[ct]
        cvec_h[:, 8 + ct] = bp_h[ct]
        cvec_h[:, 12 + ct] = gnw_h[ct]
        cvec_h[:, 16 + ct] = gnb_h[ct]
    cvec_h[:, 20:28] = indr_h

    shared = dict(w8=w8_h, wpd=wp_h, cvec=cvec_h, indb=indb_h)
    in_maps = []
    for core in range(N_CORES):
        bi, half = core // 2, core % 2
        xb = xr16[bi]
        if half:
            xb = np.ascontiguousarray(
                np.concatenate([xb[:, NQ:], xb[:, :NQ]], axis=1))
        in_maps.append({"xin": xb, **shared})
    return in_maps


def kernel(x, gn_w, gn_b, qkv_w, qkv_b, proj_w, proj_b):
    nc = _get_nc()
    in_maps = _make_in_maps(x, gn_w, gn_b, qkv_w, qkv_b, proj_w, proj_b)
    res = run_bass_kernel_spmd(nc, in_maps, list(range(N_CORES)))
    b, c, h, w = x.shape
    out_full = np.empty((b, C, N), dtype=np.float32)
    for core in range(N_CORES):
        bi, half = core // 2, core % 2
        out_full[bi, :, half * NQ:(half + 1) * NQ] = res.results[core]["out"]
    return out_full.reshape(b, c, h, w)
